# revision 19
# baseline (speedup 1.0000x reference)
"""NeuralMemory kernel for Trainium2 (8 NeuronCores, data-parallel over batch).

Computes, for B=32768, D=512:
    h   = relu(relu(key_x @ W1.T + b1) @ W2.T + b2)
    pred = h @ mem_W.T + mem_b
    resid = pred - value
    grad_W = (2/resid.size) * (resid.T @ h)
    updated_W = (1-fg) * mem_W + lr * grad_W
    out = h @ updated_W.T + mem_b

Sharding: batch B split across 8 cores (4096 rows each); weights replicated;
grad_W partial products all-reduced ([D,D] fp32, 1MB).

All matmuls run in float32r (fp32 storage, ~12-bit-mantissa PE path, full rate).
The forward pass runs in "T-space" (activations stored [D, B_tile], d on
partitions) so each layer's output directly feeds the next layer's moving
operand; key_x and the weights are pre-transposed on the host. h.T stays
resident in SBUF for the whole kernel. The grad matmul contracts over B, so h
is flipped back to natural layout with PE transposes; resid is produced in
natural layout directly (pred computed with hT as the stationary operand).
"""

import os
import sys

for _p in ("/opt/trn_rl_repo", "/root/.axon_site/_ro/trn_rl_repo"):
    if os.path.isdir(_p) and _p not in sys.path:
        sys.path.insert(0, _p)

import numpy as np

import concourse.bacc as bacc
import concourse.mybir as mybir
import concourse.tile as tile
from concourse import bass_utils, masks

dt = mybir.dt

N_CORES = 8
B = 32768
D = 512
BS = B // N_CORES          # rows per core = 4096
BT = 512                   # rows per B-tile
NT = BS // BT              # B-tiles per core = 8
NC_CH = D // 128           # 128-partition chunks per D = 4
GRAD_SCALE = 2.0 / (B * D)  # 2 / resid.size

DT_MM = dt.float32r        # matmul operand dtype (float32r | bfloat16 | float32)
WARM_LINKS = 32            # serial tiny-matmul links bridging the all-reduce

# cached compiled module + results of the last run (for test harness timing)
_NC_CACHE = None
LAST_RESULTS = None


def _build():
    nc = bacc.Bacc("TRN2", target_bir_lowering=False, debug=False,
                   num_devices=N_CORES)

    # --- per-core DRAM I/O (host pre-transposes key_x and weights) ---
    kxT = nc.dram_tensor("kxT", [D, BS], dt.float32, kind="ExternalInput")
    val = nc.dram_tensor("val", [BS, D], dt.float32, kind="ExternalInput")
    w1T = nc.dram_tensor("w1T", [D, D], dt.float32, kind="ExternalInput")
    w2T = nc.dram_tensor("w2T", [D, D], dt.float32, kind="ExternalInput")
    mwT = nc.dram_tensor("mwT", [D, D], dt.float32, kind="ExternalInput")
    b1d = nc.dram_tensor("b1", [D], dt.float32, kind="ExternalInput")
    b2d = nc.dram_tensor("b2", [D], dt.float32, kind="ExternalInput")
    mbd = nc.dram_tensor("mb", [D], dt.float32, kind="ExternalInput")
    fgd = nc.dram_tensor("fg", [1], dt.float32, kind="ExternalInput")
    lrd = nc.dram_tensor("lr", [1], dt.float32, kind="ExternalInput")
    outd = nc.dram_tensor("out", [BS, D], dt.float32, kind="ExternalOutput")

    with tile.TileContext(nc) as tc:
        with (
            tc.tile_pool(name="const", bufs=1) as cp,
            tc.tile_pool(name="wts", bufs=1) as wp,
            tc.tile_pool(name="ht", bufs=1) as hp,
            tc.tile_pool(name="io", bufs=2) as iop,
            tc.tile_pool(name="work", bufs=1) as wkp,
            tc.tile_pool(name="psg", bufs=1, space="PSUM") as psg,
            tc.tile_pool(name="psw", bufs=2, space="PSUM") as psw,
            tc.tile_pool(name="pst", bufs=2, space="PSUM") as pst,
            tc.tile_pool(name="dram", bufs=1, space="DRAM") as dramp,
        ):
            # ---- constants ----
            ident0 = cp.tile([128, 128], dt.float32, name="ident0")
            masks.make_identity(nc, ident0[:])
            ident = cp.tile([128, 128], DT_MM, name="ident")
            nc.scalar.copy(ident[:], ident0[:])

            ones0 = cp.tile([1, 128], dt.float32, name="ones0")
            nc.vector.memset(ones0[:], 1.0)

            # mem_b broadcast to [128, D] via K=1 matmul (plain fp32)
            membrow = cp.tile([1, D], dt.float32, name="membrow")
            nc.sync.dma_start(membrow[:], mbd.ap()[None, :])
            ps_mb = psw.tile([128, D], dt.float32, name="ps_mb", tag="pw")
            nc.tensor.matmul(ps_mb[:], ones0[:], membrow[:], start=True, stop=True)
            membb = cp.tile([128, D], dt.float32, name="membb")
            nc.vector.tensor_copy(membb[:], ps_mb[:])

            # biases as [128, NC_CH]: b[c*128+p] -> tile[p, c]
            # (sync queue: keep the gpsimd queue free for the big cast loads)
            b1t = cp.tile([128, NC_CH], dt.float32, name="b1t")
            nc.sync.dma_start(b1t[:], b1d.ap().rearrange("(c p) -> p c", p=128))
            b2t = cp.tile([128, NC_CH], dt.float32, name="b2t")
            nc.sync.dma_start(b2t[:], b2d.ap().rearrange("(c p) -> p c", p=128))

            # fg / lr scalars -> broadcast to [128,1] via K=1 matmul (plain fp32:
            # fp32r requires even free dims, which a [1,1] rhs violates)
            fglr = cp.tile([1, 2], dt.float32, name="fglr")
            nc.sync.dma_start(fglr[:, 0:1], fgd.ap()[None, :])
            nc.sync.dma_start(fglr[:, 1:2], lrd.ap()[None, :])
            ps_s = psw.tile([128, 2], dt.float32, name="ps_s", tag="pw")
            nc.tensor.matmul(ps_s[:, 0:2], ones0[:], fglr[:], start=True, stop=True)
            fg1m = cp.tile([128, 1], dt.float32, name="fg1m")   # 1 - fg
            nc.scalar.activation(fg1m[:], ps_s[:, 0:1],
                                 mybir.ActivationFunctionType.Copy,
                                 bias=1.0, scale=-1.0)
            lr2n = cp.tile([128, 1], dt.float32, name="lr2n")   # lr * 2/N
            nc.scalar.activation(lr2n[:], ps_s[:, 1:2],
                                 mybir.ActivationFunctionType.Copy,
                                 bias=0.0, scale=float(GRAD_SCALE))

            # ---- weights (pre-transposed on host): [128, (c, 512)] c-major ----
            # Issue order matters for the startup critical path: w1t and the
            # first key_x tile feed the very first matmuls, so they go first
            # on the gpsimd (cast-DMA) queue.
            # w1t and the first key_x tile come over the (faster) sync queue as
            # fp32 and are cast by ACT: the gpsimd cast-DMA runs well under
            # line rate, and these two gate the very first matmuls.
            w1f = wkp.tile([128, NC_CH * D], dt.float32, name="w1f", tag="ldf", bufs=2)
            nc.sync.dma_start(
                w1f[:].rearrange("p (c j) -> p c j", c=NC_CH),
                w1T.ap().rearrange("(c p) j -> p c j", p=128))
            w1t = wp.tile([128, NC_CH * D], DT_MM, name="w1t")
            nc.scalar.copy(w1t[:], w1f[:])

            kx0f = wkp.tile([128, NC_CH * BT], dt.float32, name="kx0f", tag="ldf", bufs=2)
            nc.sync.dma_start(
                kx0f[:].rearrange("p (c b) -> p c b", c=NC_CH),
                kxT.ap()[:, 0:BT].rearrange("(c p) b -> p c b", p=128))
            kx0 = iop.tile([128, NC_CH * BT], DT_MM, name="kx", tag="kx")
            nc.scalar.copy(kx0[:], kx0f[:])

            w2t = wp.tile([128, NC_CH * D], DT_MM, name="w2t")
            nc.gpsimd.dma_start(
                w2t[:].rearrange("p (c j) -> p c j", c=NC_CH),
                w2T.ap().rearrange("(c p) j -> p c j", p=128))
            mwt = wp.tile([128, NC_CH * D], DT_MM, name="mwt")
            nc.gpsimd.dma_start(
                mwt[:].rearrange("p (c j) -> p c j", c=NC_CH),
                mwT.ap().rearrange("(c p) j -> p c j", p=128))

            # ---- resident hT: NC_CH tiles [128, BS] (d_mem chunk on partitions) ----
            hT = [hp.tile([128, BS], DT_MM, name=f"hT{c}") for c in range(NC_CH)]

            # ---- grad accumulation PSUM: G[j,i] per j-chunk ----
            gps = [psg.tile([128, D], dt.float32, name=f"gps{c}")
                   for c in range(NC_CH)]

            # =================== pass 1 over B-tiles ===================
            for t in range(NT):
                b0 = t * BT
                if t == 0:
                    kx = kx0
                else:
                    kx = iop.tile([128, NC_CH * BT], DT_MM, name="kx", tag="kx")
                    nc.gpsimd.dma_start(
                        kx[:].rearrange("p (c b) -> p c b", c=NC_CH),
                        kxT.ap()[:, b0:b0 + BT].rearrange("(c p) b -> p c b", p=128))
                vt = iop.tile([128, NC_CH * D], dt.bfloat16, name="vt", tag="vt")
                nc.gpsimd.dma_start(
                    vt[:].rearrange("p (c i) -> p c i", c=NC_CH),
                    val.ap()[b0:b0 + BT, :].rearrange("(c p) i -> p c i", p=128))

                # M1: h1T = relu(W1T . kxT + b1)   [j1 on partitions, b free]
                h1 = wkp.tile([128, NC_CH * BT], DT_MM, name="h1", tag="h1")
                for jc in range(NC_CH):
                    pw = psw.tile([128, BT], dt.float32, name="pw_m1", tag="pw")
                    for kc in range(NC_CH):
                        nc.tensor.matmul(
                            pw[:],
                            w1t[:, kc * D + jc * 128: kc * D + (jc + 1) * 128],
                            kx[:, kc * BT:(kc + 1) * BT],
                            start=(kc == 0), stop=(kc == NC_CH - 1))
                    nc.scalar.activation(
                        h1[:, jc * BT:(jc + 1) * BT], pw[:],
                        mybir.ActivationFunctionType.Relu,
                        bias=b1t[:, jc:jc + 1], scale=1.0)

                # M2: hT = relu(W2T . h1T + b2) -> resident
                for jc in range(NC_CH):
                    pw = psw.tile([128, BT], dt.float32, name="pw_m2", tag="pw")
                    for kc in range(NC_CH):
                        nc.tensor.matmul(
                            pw[:],
                            w2t[:, kc * D + jc * 128: kc * D + (jc + 1) * 128],
                            h1[:, kc * BT:(kc + 1) * BT],
                            start=(kc == 0), stop=(kc == NC_CH - 1))
                    nc.scalar.activation(
                        hT[jc][:, b0:b0 + BT], pw[:],
                        mybir.ActivationFunctionType.Relu,
                        bias=b2t[:, jc:jc + 1], scale=1.0)

                # M3: pred (natural) = hT.T . mem_WT ; resid = pred - (value-mem_b)
                # (mem_b is folded into value on the host)
                resid = wkp.tile([128, NC_CH * D], DT_MM, name="resid", tag="resid")
                for bs in range(NC_CH):
                    pw = psw.tile([128, D], dt.float32, name="pw_m3", tag="pw")
                    for jc in range(NC_CH):
                        nc.tensor.matmul(
                            pw[:],
                            hT[jc][:, b0 + bs * 128: b0 + (bs + 1) * 128],
                            mwt[:, jc * D:(jc + 1) * D],
                            start=(jc == 0), stop=(jc == NC_CH - 1))
                    nc.vector.tensor_sub(
                        resid[:, bs * D:(bs + 1) * D], pw[:],
                        vt[:, bs * D:(bs + 1) * D])

                # transpose hT -> h natural (PE), then M4: G += h_nat.T-chunks
                for bs in range(NC_CH):
                    pt = pst.tile([128, D], DT_MM, name="pt", tag="pt")
                    for jc in range(NC_CH):
                        nc.tensor.transpose(
                            pt[:, jc * 128:(jc + 1) * 128],
                            hT[jc][:, b0 + bs * 128: b0 + (bs + 1) * 128],
                            ident[:])
                    hn = wkp.tile([128, D], DT_MM, name="hn", tag="hn")
                    nc.vector.tensor_copy(hn[:], pt[:])
                    first = (t == 0 and bs == 0)
                    last = (t == NT - 1 and bs == NC_CH - 1)
                    for jc in range(NC_CH):
                        nc.tensor.matmul(
                            gps[jc][:],
                            hn[:, jc * 128:(jc + 1) * 128],
                            resid[:, bs * D:(bs + 1) * D],
                            start=first, stop=last)

            # =================== all-reduce G (bf16 wire format) ===================
            gsb = wkp.tile([128, NC_CH * D], dt.bfloat16, name="gsb", tag="gsb")
            for jc in range(NC_CH):
                nc.vector.tensor_copy(gsb[:, jc * D:(jc + 1) * D], gps[jc][:])
            cin = dramp.tile([D, D], dt.bfloat16, name="cin")
            cout = dramp.tile([D, D], dt.bfloat16, name="cout", addr_space="Shared")
            nc.sync.dma_start(
                cin[:].rearrange("(c p) i -> p c i", p=128),
                gsb[:].rearrange("p (c i) -> p c i", c=NC_CH))
            nc.gpsimd.collective_compute(
                "AllReduce", mybir.AluOpType.add,
                replica_groups=[list(range(N_CORES))],
                ins=[cin.opt()], outs=[cout.opt()])
            gts = wkp.tile([128, NC_CH * D], dt.bfloat16, name="gts", tag="gts")
            nc.sync.dma_start(
                gts[:].rearrange("p (c i) -> p c i", c=NC_CH),
                cout[:].rearrange("(c p) i -> p c i", p=128))

            # ====== pass 2a (overlaps the all-reduce): otA = h @ (lr*s*G_local).T
            # + mem_b.  out = h @ uW.T + mem_b splits into an AR-independent
            # local-gradient part and a remainder using G_total - G_local;
            # the local part fills the PE during the collective (also keeping
            # the HAM clock gate warm).
            uwta = wkp.tile([128, NC_CH * D], DT_MM, name="uwta", tag="uwa")
            nc.vector.tensor_scalar(uwta[:], gsb[:], lr2n[:], None,
                                    mybir.AluOpType.mult)
            otA = [hp.tile([128, NC_CH * D], dt.float32, name=f"otA{t}")
                   for t in range(NT)]
            for t in range(NT):
                b0 = t * BT
                for bs in range(NC_CH):
                    pw = psw.tile([128, D], dt.float32, name="pw_m5a", tag="pw")
                    for jc in range(NC_CH):
                        nc.tensor.matmul(
                            pw[:],
                            hT[jc][:, b0 + bs * 128: b0 + (bs + 1) * 128],
                            uwta[:, jc * D:(jc + 1) * D],
                            start=(jc == 0), stop=(jc == NC_CH - 1))
                    nc.vector.tensor_add(otA[t][:, bs * D:(bs + 1) * D], pw[:],
                                         membb[:])

            # Dense PE warm-up burst gated on the all-reduce result, in case
            # the PE clock still throttled during any residual idle.
            wb_ps = pst.tile([128, D], dt.float32, name="wb_ps", tag="pt")
            nc.tensor.matmul(wb_ps[:], w1t[:, 0:128], gts[:, 0:D],
                             start=True, stop=False)
            for wi in range(7):
                nc.tensor.matmul(wb_ps[:], w1t[:, 0:128], w1t[:, 0:D],
                                 start=False, stop=(wi == 6))

            # remainder weights: uWT_b = (1-fg)*mem_WT + (lr*2/N)*(G_tot-G_loc)
            uwd = wkp.tile([128, NC_CH * D], DT_MM, name="uwd", tag="uwd")
            nc.vector.tensor_sub(uwd[:], gts[:], gsb[:])
            uwa = wkp.tile([128, NC_CH * D], DT_MM, name="uwa", tag="uwa2")
            nc.vector.tensor_scalar(uwa[:], uwd[:], lr2n[:], None,
                                    mybir.AluOpType.mult)
            uwt = wp.tile([128, NC_CH * D], DT_MM, name="uwt")
            nc.vector.scalar_tensor_tensor(
                uwt[:], mwt[:], fg1m[:], uwa[:],
                mybir.AluOpType.mult, mybir.AluOpType.add)

            # ========== pass 2b: out = otA + h @ uWT_b.T ==========
            for t in range(NT):
                b0 = t * BT
                ot = iop.tile([128, NC_CH * D], dt.float32, name="ot", tag="ot")
                for bs in range(NC_CH):
                    pw = psw.tile([128, D], dt.float32, name="pw_m5", tag="pw")
                    for jc in range(NC_CH):
                        nc.tensor.matmul(
                            pw[:],
                            hT[jc][:, b0 + bs * 128: b0 + (bs + 1) * 128],
                            uwt[:, jc * D:(jc + 1) * D],
                            start=(jc == 0), stop=(jc == NC_CH - 1))
                    nc.vector.tensor_add(ot[:, bs * D:(bs + 1) * D], pw[:],
                                         otA[t][:, bs * D:(bs + 1) * D])
                nc.sync.dma_start(
                    outd.ap()[b0:b0 + BT, :].rearrange("(c p) i -> p c i", p=128),
                    ot[:].rearrange("p (c i) -> p c i", c=NC_CH))

    nc.compile()
    return nc


def _get_nc():
    global _NC_CACHE
    if _NC_CACHE is None:
        _NC_CACHE = _build()
    return _NC_CACHE


def kernel(key_x, value, W1, b1, W2, b2, mem_W, mem_b, forgetting_gate,
           learning_rate):
    global LAST_RESULTS
    key_x = np.ascontiguousarray(np.asarray(key_x, dtype=np.float32))
    value = np.ascontiguousarray(np.asarray(value, dtype=np.float32))
    w1T = np.ascontiguousarray(np.asarray(W1, dtype=np.float32).T)
    w2T = np.ascontiguousarray(np.asarray(W2, dtype=np.float32).T)
    mwT = np.ascontiguousarray(np.asarray(mem_W, dtype=np.float32).T)
    b1 = np.ascontiguousarray(np.asarray(b1, dtype=np.float32))
    b2 = np.ascontiguousarray(np.asarray(b2, dtype=np.float32))
    mem_b = np.ascontiguousarray(np.asarray(mem_b, dtype=np.float32))
    fg = np.ascontiguousarray(np.asarray(forgetting_gate, dtype=np.float32))
    lr = np.ascontiguousarray(np.asarray(learning_rate, dtype=np.float32))

    # mem_b is folded into value on the host: the kernel computes
    # resid = pred_nobias - (value - mem_b) == (pred_nobias + mem_b) - value
    value_adj = value - mem_b[None, :]

    in_maps = []
    for c in range(N_CORES):
        rows = slice(c * BS, (c + 1) * BS)
        in_maps.append({
            "kxT": np.ascontiguousarray(key_x[rows, :].T),
            "val": value_adj[rows, :],
            "w1T": w1T, "w2T": w2T, "mwT": mwT,
            "b1": b1, "b2": b2, "mb": mem_b, "fg": fg, "lr": lr,
        })

    nc = _get_nc()
    LAST_RESULTS = bass_utils.run_bass_kernel_spmd(
        nc, in_maps, core_ids=list(range(N_CORES)))
    out = np.concatenate([LAST_RESULTS.results[c]["out"]
                          for c in range(N_CORES)], axis=0)
    return out


if __name__ == "__main__":
    rng = np.random.default_rng(0)
    kx = rng.standard_normal((B, D)).astype(np.float32)
    vv = rng.standard_normal((B, D)).astype(np.float32)
    s = 1.0 / np.sqrt(D)
    W1 = rng.uniform(-s, s, (D, D)).astype(np.float32)
    b1 = rng.uniform(-s, s, (D,)).astype(np.float32)
    W2 = rng.uniform(-s, s, (D, D)).astype(np.float32)
    b2 = rng.uniform(-s, s, (D,)).astype(np.float32)
    mW = rng.uniform(-s, s, (D, D)).astype(np.float32)
    mb = rng.uniform(-s, s, (D,)).astype(np.float32)
    fg = np.ones((1,), np.float32)
    lr = np.ones((1,), np.float32)

    h = np.maximum(kx @ W1.T + b1, 0)
    h = np.maximum(h @ W2.T + b2, 0)
    pred = h @ mW.T + mb
    resid = pred - vv
    grad = (2.0 / resid.size) * (resid.T @ h)
    uW = (1 - fg) * mW + lr * grad
    ref = h @ uW.T + mb

    out = kernel(kx, vv, W1, b1, W2, b2, mW, mb, fg, lr)
    d = np.abs(out - ref)
    print("max abs err:", d.max(), "max rel:", d.max() / np.abs(ref).max())


# revision 21
# speedup vs baseline: 1.0485x; 1.0485x over previous
"""NeuralMemory kernel for Trainium2 (8 NeuronCores, data-parallel over batch).

Computes, for B=32768, D=512:
    h   = relu(relu(key_x @ W1.T + b1) @ W2.T + b2)
    pred = h @ mem_W.T + mem_b
    resid = pred - value
    grad_W = (2/resid.size) * (resid.T @ h)
    updated_W = (1-fg) * mem_W + lr * grad_W
    out = h @ updated_W.T + mem_b

Sharding: batch B split across 8 cores (4096 rows each); weights replicated;
grad_W partial products all-reduced ([D,D] fp32, 1MB).

All matmuls run in float32r (fp32 storage, ~12-bit-mantissa PE path, full rate).
The forward pass runs in "T-space" (activations stored [D, B_tile], d on
partitions) so each layer's output directly feeds the next layer's moving
operand; key_x and the weights are pre-transposed on the host. h.T stays
resident in SBUF for the whole kernel. The grad matmul contracts over B, so h
is flipped back to natural layout with PE transposes; resid is produced in
natural layout directly (pred computed with hT as the stationary operand).
"""

import os
import sys

for _p in ("/opt/trn_rl_repo", "/root/.axon_site/_ro/trn_rl_repo"):
    if os.path.isdir(_p) and _p not in sys.path:
        sys.path.insert(0, _p)

import numpy as np

import concourse.bacc as bacc
import concourse.mybir as mybir
import concourse.tile as tile
from concourse import bass_utils, masks

dt = mybir.dt

N_CORES = 8
B = 32768
D = 512
BS = B // N_CORES          # rows per core = 4096
BT = 512                   # rows per B-tile
NT = BS // BT              # B-tiles per core = 8
NC_CH = D // 128           # 128-partition chunks per D = 4
GRAD_SCALE = 2.0 / (B * D)  # 2 / resid.size

DT_MM = dt.float32r        # matmul operand dtype (float32r | bfloat16 | float32)
WARM_LINKS = 32            # serial tiny-matmul links bridging the all-reduce

# cached compiled module + results of the last run (for test harness timing)
_NC_CACHE = None
LAST_RESULTS = None


def _build():
    nc = bacc.Bacc("TRN2", target_bir_lowering=False, debug=False,
                   num_devices=N_CORES)

    # --- per-core DRAM I/O (host pre-transposes key_x and weights) ---
    kxT = nc.dram_tensor("kxT", [D, BS], dt.float32, kind="ExternalInput")
    val = nc.dram_tensor("val", [BS, D], dt.float32, kind="ExternalInput")
    w1T = nc.dram_tensor("w1T", [D, D], dt.float32, kind="ExternalInput")
    w2T = nc.dram_tensor("w2T", [D, D], dt.float32, kind="ExternalInput")
    mwT = nc.dram_tensor("mwT", [D, D], dt.float32, kind="ExternalInput")
    b1d = nc.dram_tensor("b1", [D], dt.float32, kind="ExternalInput")
    b2d = nc.dram_tensor("b2", [D], dt.float32, kind="ExternalInput")
    mbd = nc.dram_tensor("mb", [D], dt.float32, kind="ExternalInput")
    fgd = nc.dram_tensor("fg", [1], dt.float32, kind="ExternalInput")
    lrd = nc.dram_tensor("lr", [1], dt.float32, kind="ExternalInput")
    outd = nc.dram_tensor("out", [BS, D], dt.float32, kind="ExternalOutput")

    with tile.TileContext(nc) as tc:
        with (
            tc.tile_pool(name="const", bufs=1) as cp,
            tc.tile_pool(name="wts", bufs=1) as wp,
            tc.tile_pool(name="ht", bufs=1) as hp,
            tc.tile_pool(name="io", bufs=2) as iop,
            tc.tile_pool(name="work", bufs=1) as wkp,
            tc.tile_pool(name="psg", bufs=1, space="PSUM") as psg,
            tc.tile_pool(name="psw", bufs=2, space="PSUM") as psw,
            tc.tile_pool(name="pst", bufs=2, space="PSUM") as pst,
            tc.tile_pool(name="dram", bufs=1, space="DRAM") as dramp,
        ):
            # ---- constants ----
            ident0 = cp.tile([128, 128], dt.float32, name="ident0")
            masks.make_identity(nc, ident0[:])
            ident = cp.tile([128, 128], DT_MM, name="ident")
            nc.scalar.copy(ident[:], ident0[:])

            ones0 = cp.tile([1, 128], dt.float32, name="ones0")
            nc.vector.memset(ones0[:], 1.0)

            # mem_b broadcast to [128, D] via K=1 matmul (plain fp32)
            membrow = cp.tile([1, D], dt.float32, name="membrow")
            nc.sync.dma_start(membrow[:], mbd.ap()[None, :])
            ps_mb = psw.tile([128, D], dt.float32, name="ps_mb", tag="pw")
            nc.tensor.matmul(ps_mb[:], ones0[:], membrow[:], start=True, stop=True)
            membb = cp.tile([128, D], dt.float32, name="membb")
            nc.vector.tensor_copy(membb[:], ps_mb[:])

            # biases as [128, NC_CH]: b[c*128+p] -> tile[p, c]
            # (sync queue: keep the gpsimd queue free for the big cast loads)
            b1t = cp.tile([128, NC_CH], dt.float32, name="b1t")
            nc.sync.dma_start(b1t[:], b1d.ap().rearrange("(c p) -> p c", p=128))
            b2t = cp.tile([128, NC_CH], dt.float32, name="b2t")
            nc.sync.dma_start(b2t[:], b2d.ap().rearrange("(c p) -> p c", p=128))

            # fg / lr scalars -> broadcast to [128,1] via K=1 matmul (plain fp32:
            # fp32r requires even free dims, which a [1,1] rhs violates)
            fglr = cp.tile([1, 2], dt.float32, name="fglr")
            nc.sync.dma_start(fglr[:, 0:1], fgd.ap()[None, :])
            nc.sync.dma_start(fglr[:, 1:2], lrd.ap()[None, :])
            ps_s = psw.tile([128, 2], dt.float32, name="ps_s", tag="pw")
            nc.tensor.matmul(ps_s[:, 0:2], ones0[:], fglr[:], start=True, stop=True)
            fg1m = cp.tile([128, 1], dt.float32, name="fg1m")   # 1 - fg
            nc.scalar.activation(fg1m[:], ps_s[:, 0:1],
                                 mybir.ActivationFunctionType.Copy,
                                 bias=1.0, scale=-1.0)
            lr2n = cp.tile([128, 1], dt.float32, name="lr2n")   # lr * 2/N
            nc.scalar.activation(lr2n[:], ps_s[:, 1:2],
                                 mybir.ActivationFunctionType.Copy,
                                 bias=0.0, scale=float(GRAD_SCALE))

            # ---- weights (pre-transposed on host): [128, (c, 512)] c-major ----
            # Issue order matters for the startup critical path: w1t and the
            # first key_x tile feed the very first matmuls, so they go first
            # on the gpsimd (cast-DMA) queue.
            w1t = wp.tile([128, NC_CH * D], DT_MM, name="w1t")
            nc.gpsimd.dma_start(
                w1t[:].rearrange("p (c j) -> p c j", c=NC_CH),
                w1T.ap().rearrange("(c p) j -> p c j", p=128))

            kx0 = iop.tile([128, NC_CH * BT], DT_MM, name="kx", tag="kx")
            nc.gpsimd.dma_start(
                kx0[:].rearrange("p (c b) -> p c b", c=NC_CH),
                kxT.ap()[:, 0:BT].rearrange("(c p) b -> p c b", p=128))

            w2t = wp.tile([128, NC_CH * D], DT_MM, name="w2t")
            nc.gpsimd.dma_start(
                w2t[:].rearrange("p (c j) -> p c j", c=NC_CH),
                w2T.ap().rearrange("(c p) j -> p c j", p=128))
            mwt = wp.tile([128, NC_CH * D], DT_MM, name="mwt")
            nc.gpsimd.dma_start(
                mwt[:].rearrange("p (c j) -> p c j", c=NC_CH),
                mwT.ap().rearrange("(c p) j -> p c j", p=128))

            # ---- resident hT: NC_CH tiles [128, BS] (d_mem chunk on partitions) ----
            hT = [hp.tile([128, BS], DT_MM, name=f"hT{c}") for c in range(NC_CH)]

            # ---- grad accumulation PSUM: G[j,i] per j-chunk ----
            gps = [psg.tile([128, D], dt.float32, name=f"gps{c}")
                   for c in range(NC_CH)]

            # =================== pass 1 over B-tiles ===================
            for t in range(NT):
                b0 = t * BT
                if t == 0:
                    kx = kx0
                else:
                    kx = iop.tile([128, NC_CH * BT], DT_MM, name="kx", tag="kx")
                    nc.gpsimd.dma_start(
                        kx[:].rearrange("p (c b) -> p c b", c=NC_CH),
                        kxT.ap()[:, b0:b0 + BT].rearrange("(c p) b -> p c b", p=128))
                vt = iop.tile([128, NC_CH * D], dt.bfloat16, name="vt", tag="vt")
                nc.gpsimd.dma_start(
                    vt[:].rearrange("p (c i) -> p c i", c=NC_CH),
                    val.ap()[b0:b0 + BT, :].rearrange("(c p) i -> p c i", p=128))

                # M1: h1T = relu(W1T . kxT + b1)   [j1 on partitions, b free]
                h1 = wkp.tile([128, NC_CH * BT], DT_MM, name="h1", tag="h1")
                for jc in range(NC_CH):
                    pw = psw.tile([128, BT], dt.float32, name="pw_m1", tag="pw")
                    for kc in range(NC_CH):
                        nc.tensor.matmul(
                            pw[:],
                            w1t[:, kc * D + jc * 128: kc * D + (jc + 1) * 128],
                            kx[:, kc * BT:(kc + 1) * BT],
                            start=(kc == 0), stop=(kc == NC_CH - 1))
                    nc.scalar.activation(
                        h1[:, jc * BT:(jc + 1) * BT], pw[:],
                        mybir.ActivationFunctionType.Relu,
                        bias=b1t[:, jc:jc + 1], scale=1.0)

                # M2: hT = relu(W2T . h1T + b2) -> resident
                for jc in range(NC_CH):
                    pw = psw.tile([128, BT], dt.float32, name="pw_m2", tag="pw")
                    for kc in range(NC_CH):
                        nc.tensor.matmul(
                            pw[:],
                            w2t[:, kc * D + jc * 128: kc * D + (jc + 1) * 128],
                            h1[:, kc * BT:(kc + 1) * BT],
                            start=(kc == 0), stop=(kc == NC_CH - 1))
                    nc.scalar.activation(
                        hT[jc][:, b0:b0 + BT], pw[:],
                        mybir.ActivationFunctionType.Relu,
                        bias=b2t[:, jc:jc + 1], scale=1.0)

                # M3: pred (natural) = hT.T . mem_WT ; resid = pred - (value-mem_b)
                # (mem_b is folded into value on the host)
                resid = wkp.tile([128, NC_CH * D], DT_MM, name="resid", tag="resid")
                for bs in range(NC_CH):
                    pw = psw.tile([128, D], dt.float32, name="pw_m3", tag="pw")
                    for jc in range(NC_CH):
                        nc.tensor.matmul(
                            pw[:],
                            hT[jc][:, b0 + bs * 128: b0 + (bs + 1) * 128],
                            mwt[:, jc * D:(jc + 1) * D],
                            start=(jc == 0), stop=(jc == NC_CH - 1))
                    nc.vector.tensor_sub(
                        resid[:, bs * D:(bs + 1) * D], pw[:],
                        vt[:, bs * D:(bs + 1) * D])

                # transpose hT -> h natural (PE), then M4: G += h_nat.T-chunks
                for bs in range(NC_CH):
                    pt = pst.tile([128, D], DT_MM, name="pt", tag="pt")
                    for jc in range(NC_CH):
                        nc.tensor.transpose(
                            pt[:, jc * 128:(jc + 1) * 128],
                            hT[jc][:, b0 + bs * 128: b0 + (bs + 1) * 128],
                            ident[:])
                    hn = wkp.tile([128, D], DT_MM, name="hn", tag="hn")
                    nc.vector.tensor_copy(hn[:], pt[:])
                    first = (t == 0 and bs == 0)
                    last = (t == NT - 1 and bs == NC_CH - 1)
                    for jc in range(NC_CH):
                        nc.tensor.matmul(
                            gps[jc][:],
                            hn[:, jc * 128:(jc + 1) * 128],
                            resid[:, bs * D:(bs + 1) * D],
                            start=first, stop=last)

            # =================== all-reduce G (bf16 wire format) ===================
            gsb = wkp.tile([128, NC_CH * D], dt.bfloat16, name="gsb", tag="gsb")
            for jc in range(NC_CH):
                nc.vector.tensor_copy(gsb[:, jc * D:(jc + 1) * D], gps[jc][:])
            cin = dramp.tile([D, D], dt.bfloat16, name="cin")
            cout = dramp.tile([D, D], dt.bfloat16, name="cout", addr_space="Shared")
            nc.sync.dma_start(
                cin[:].rearrange("(c p) i -> p c i", p=128),
                gsb[:].rearrange("p (c i) -> p c i", c=NC_CH))
            nc.gpsimd.collective_compute(
                "AllReduce", mybir.AluOpType.add,
                replica_groups=[list(range(N_CORES))],
                ins=[cin.opt()], outs=[cout.opt()])
            gts = wkp.tile([128, NC_CH * D], dt.bfloat16, name="gts", tag="gts")
            nc.sync.dma_start(
                gts[:].rearrange("p (c i) -> p c i", c=NC_CH),
                cout[:].rearrange("(c p) i -> p c i", p=128))

            # ====== pass 2a (overlaps the all-reduce): otA = h @ (lr*s*G_local).T
            # + mem_b.  out = h @ uW.T + mem_b splits into an AR-independent
            # local-gradient part and a remainder using G_total - G_local;
            # the local part fills the PE during the collective (also keeping
            # the HAM clock gate warm).
            uwta = wkp.tile([128, NC_CH * D], DT_MM, name="uwta", tag="uwa")
            nc.vector.tensor_scalar(uwta[:], gsb[:], lr2n[:], None,
                                    mybir.AluOpType.mult)
            otA = [hp.tile([128, NC_CH * D], dt.float32, name=f"otA{t}")
                   for t in range(NT)]
            for t in range(NT):
                b0 = t * BT
                for bs in range(NC_CH):
                    pool = psw if bs % 2 == 0 else pst
                    pw = pool.tile([128, D], dt.float32, name="pw_m5a",
                                   tag="pw" if bs % 2 == 0 else "pt")
                    for jc in range(NC_CH):
                        nc.tensor.matmul(
                            pw[:],
                            hT[jc][:, b0 + bs * 128: b0 + (bs + 1) * 128],
                            uwta[:, jc * D:(jc + 1) * D],
                            start=(jc == 0), stop=(jc == NC_CH - 1))
                    nc.vector.tensor_add(otA[t][:, bs * D:(bs + 1) * D], pw[:],
                                         membb[:])

            # Dense PE warm-up burst gated on the all-reduce result, in case
            # the PE clock still throttled during any residual idle.
            wb_ps = pst.tile([128, D], dt.float32, name="wb_ps", tag="pt")
            nc.tensor.matmul(wb_ps[:], w1t[:, 0:128], gts[:, 0:D],
                             start=True, stop=False)
            for wi in range(7):
                nc.tensor.matmul(wb_ps[:], w1t[:, 0:128], w1t[:, 0:D],
                                 start=False, stop=(wi == 6))

            # remainder weights: uWT_b = (1-fg)*mem_WT + (lr*2/N)*(G_tot-G_loc)
            uwd = wkp.tile([128, NC_CH * D], DT_MM, name="uwd", tag="uwd")
            nc.vector.tensor_sub(uwd[:], gts[:], gsb[:])
            uwa = wkp.tile([128, NC_CH * D], DT_MM, name="uwa", tag="uwa2")
            nc.vector.tensor_scalar(uwa[:], uwd[:], lr2n[:], None,
                                    mybir.AluOpType.mult)
            uwt = wp.tile([128, NC_CH * D], DT_MM, name="uwt")
            nc.vector.scalar_tensor_tensor(
                uwt[:], mwt[:], fg1m[:], uwa[:],
                mybir.AluOpType.mult, mybir.AluOpType.add)

            # ========== pass 2b: out = otA + h @ uWT_b.T ==========
            for t in range(NT):
                b0 = t * BT
                ot = iop.tile([128, NC_CH * D], dt.float32, name="ot", tag="ot")
                for bs in range(NC_CH):
                    pool = psw if bs % 2 == 0 else pst
                    pw = pool.tile([128, D], dt.float32, name="pw_m5",
                                   tag="pw" if bs % 2 == 0 else "pt")
                    for jc in range(NC_CH):
                        nc.tensor.matmul(
                            pw[:],
                            hT[jc][:, b0 + bs * 128: b0 + (bs + 1) * 128],
                            uwt[:, jc * D:(jc + 1) * D],
                            start=(jc == 0), stop=(jc == NC_CH - 1))
                    nc.vector.tensor_add(ot[:, bs * D:(bs + 1) * D], pw[:],
                                         otA[t][:, bs * D:(bs + 1) * D])
                nc.sync.dma_start(
                    outd.ap()[b0:b0 + BT, :].rearrange("(c p) i -> p c i", p=128),
                    ot[:].rearrange("p (c i) -> p c i", c=NC_CH))

    nc.compile()
    return nc


def _get_nc():
    global _NC_CACHE
    if _NC_CACHE is None:
        _NC_CACHE = _build()
    return _NC_CACHE


def kernel(key_x, value, W1, b1, W2, b2, mem_W, mem_b, forgetting_gate,
           learning_rate):
    global LAST_RESULTS
    key_x = np.ascontiguousarray(np.asarray(key_x, dtype=np.float32))
    value = np.ascontiguousarray(np.asarray(value, dtype=np.float32))
    w1T = np.ascontiguousarray(np.asarray(W1, dtype=np.float32).T)
    w2T = np.ascontiguousarray(np.asarray(W2, dtype=np.float32).T)
    mwT = np.ascontiguousarray(np.asarray(mem_W, dtype=np.float32).T)
    b1 = np.ascontiguousarray(np.asarray(b1, dtype=np.float32))
    b2 = np.ascontiguousarray(np.asarray(b2, dtype=np.float32))
    mem_b = np.ascontiguousarray(np.asarray(mem_b, dtype=np.float32))
    fg = np.ascontiguousarray(np.asarray(forgetting_gate, dtype=np.float32))
    lr = np.ascontiguousarray(np.asarray(learning_rate, dtype=np.float32))

    # mem_b is folded into value on the host: the kernel computes
    # resid = pred_nobias - (value - mem_b) == (pred_nobias + mem_b) - value
    value_adj = value - mem_b[None, :]

    in_maps = []
    for c in range(N_CORES):
        rows = slice(c * BS, (c + 1) * BS)
        in_maps.append({
            "kxT": np.ascontiguousarray(key_x[rows, :].T),
            "val": value_adj[rows, :],
            "w1T": w1T, "w2T": w2T, "mwT": mwT,
            "b1": b1, "b2": b2, "mb": mem_b, "fg": fg, "lr": lr,
        })

    nc = _get_nc()
    LAST_RESULTS = bass_utils.run_bass_kernel_spmd(
        nc, in_maps, core_ids=list(range(N_CORES)))
    out = np.concatenate([LAST_RESULTS.results[c]["out"]
                          for c in range(N_CORES)], axis=0)
    return out


if __name__ == "__main__":
    rng = np.random.default_rng(0)
    kx = rng.standard_normal((B, D)).astype(np.float32)
    vv = rng.standard_normal((B, D)).astype(np.float32)
    s = 1.0 / np.sqrt(D)
    W1 = rng.uniform(-s, s, (D, D)).astype(np.float32)
    b1 = rng.uniform(-s, s, (D,)).astype(np.float32)
    W2 = rng.uniform(-s, s, (D, D)).astype(np.float32)
    b2 = rng.uniform(-s, s, (D,)).astype(np.float32)
    mW = rng.uniform(-s, s, (D, D)).astype(np.float32)
    mb = rng.uniform(-s, s, (D,)).astype(np.float32)
    fg = np.ones((1,), np.float32)
    lr = np.ones((1,), np.float32)

    h = np.maximum(kx @ W1.T + b1, 0)
    h = np.maximum(h @ W2.T + b2, 0)
    pred = h @ mW.T + mb
    resid = pred - vv
    grad = (2.0 / resid.size) * (resid.T @ h)
    uW = (1 - fg) * mW + lr * grad
    ref = h @ uW.T + mb

    out = kernel(kx, vv, W1, b1, W2, b2, mW, mb, fg, lr)
    d = np.abs(out - ref)
    print("max abs err:", d.max(), "max rel:", d.max() / np.abs(ref).max())
